# revision 1
# baseline (speedup 1.0000x reference)
"""MoE SwiGLU experts (T=2048, H=2048, I=5632, E=8, top-2) on 8 trn2 cores.

Strategy: expert-parallel routed compute. The reference computes all 8
experts densely for every token, but the output only needs each token's
top-2 experts, so we gather tokens per expert on the host (merging the
case where both top-k slots pick the same expert), run one expert per
NeuronCore on its ~T*K/E gathered tokens, and scatter-combine with the
router weights on the host.  4x less device FLOPs than dense.

Per core (expert e), with C = padded token capacity:
  phase 1: hT[i, c] = silu(w1[e].T @ xgT) * (w3[e].T @ xgT)   [I, C]
           - accumulate over 16 H-chunks of 128 in PSUM, f32r matmuls
  phase 2: y[c, h]  = hT.T @ w2[e]                            [C, H]
           - accumulate over 44 I-chunks of 128 in PSUM
All matmuls use float32r (full PE rate at moving-dim >= 256, ~1.4e-4
max rel err vs fp32).  Weights are host-retiled so every DMA is one
contiguous [128, wtile] block (2KB/partition lines when C <= 512).
"""

import numpy as np

import concourse.bacc as bacc
import concourse.mybir as mybir
import concourse.tile as tile
from concourse.bass_utils import run_bass_kernel_spmd

E = 8
H = 2048
I = 5632
HK = H // 128   # 16 contraction chunks for phase 1
IK = I // 128   # 44 contraction chunks for phase 2
HG = H // 512   # 4 output column groups (w2)
C_CAP = 640     # max tokens per expert per round (SBUF budget)

F32 = mybir.dt.float32
F32R = mybir.dt.float32r
SILU = mybir.ActivationFunctionType.Silu

_prog_cache: dict[int, object] = {}


def _chunk_list(c):
    """Split c (multiple of 128) into moving-dim chunks of at most 512
    (PSUM bank), preferring >=256 so f32r matmuls run at full rate."""
    out, off, r = [], 0, c
    while r > 0:
        if r <= 512:
            t = r
        elif r < 768:
            t = r - 256
        else:
            t = 512
        out.append((off, t))
        off += t
        r -= t
    return out


def _wtile(c):
    """Phase-1 weight tile width: 2 weights x (wtile/128) ic-tiles x
    n_chunks accumulation groups must fit in 8 PSUM banks."""
    return 512 if len(_chunk_list(c)) == 1 else 256


def _build(c):
    nc = bacc.Bacc("TRN2", target_bir_lowering=False, debug=False, num_devices=E)
    wt_w = _wtile(c)
    icpt = wt_w // 128          # ic-tiles per phase-1 weight tile
    n_icg = I // wt_w           # phase-1 weight groups
    xgT = nc.dram_tensor("xgT", [HK, 128, c], F32R, kind="ExternalInput")
    w1 = nc.dram_tensor(
        "w1", [n_icg, HK // 2, 128, 2, wt_w], F32R, kind="ExternalInput"
    )
    w3 = nc.dram_tensor(
        "w3", [n_icg, HK // 2, 128, 2, wt_w], F32R, kind="ExternalInput"
    )
    w2 = nc.dram_tensor("w2", [HG, IK, 128, 512], F32R, kind="ExternalInput")
    y = nc.dram_tensor("y", [c, H], F32, kind="ExternalOutput")
    scratch = nc.dram_tensor("scratch", [128, 512], F32, kind="ExternalOutput")

    ch = _chunk_list(c)
    tt_n = c // 128
    wbufs = 6

    with tile.TileContext(nc) as tc:
        with (
            tc.tile_pool(name="xg", bufs=1) as xpool,
            tc.tile_pool(name="h", bufs=1) as hpool,
            tc.tile_pool(name="w", bufs=wbufs) as wpool,
            tc.tile_pool(name="w2p", bufs=8) as w2pool,
            tc.tile_pool(name="ps", bufs=8, space="PSUM") as pspool,
            tc.tile_pool(name="o", bufs=4) as opool,
        ):
            # PE warmup: matmuls on a zeroed tile keep the PE busy (and the
            # HAM clock un-throttled) while the first input DMAs land.
            wu0 = xpool.tile([128, 512], F32, tag="wu0", name="wu0")
            nc.vector.memset(wu0[:], 0.0)
            wu = xpool.tile([128, 512], F32R, tag="wu", name="wu")
            nc.vector.tensor_copy(wu[:], wu0[:])
            wups = pspool.tile([128, 512], F32, tag="ps", name="wups")
            for _ in range(48):
                nc.tensor.matmul(wups[:], wu[:, :128], wu[:], start=True, stop=True)
            wuo = opool.tile([128, 512], F32, tag="o", name="wuo")
            nc.vector.tensor_copy(wuo[:], wups[:])
            nc.sync.dma_start(scratch[:], wuo[:])

            # Gathered tokens: first tiles partition-split across queues so
            # the first real matmul chain starts ASAP.
            xg = []
            for hk in range(HK):
                t = xpool.tile([128, c], F32R, tag=f"xg{hk}", name=f"xg{hk}")
                if hk < 4:
                    nc.sync.dma_start(t[0:64, :], xgT[hk, 0:64])
                    nc.sync.dma_start(t[64:128, :], xgT[hk, 64:128])
                else:
                    nc.sync.dma_start(t[:], xgT[hk])
                xg.append(t)
            hT = [
                hpool.tile([128, c], F32R, tag=f"h{ik}", name=f"h{ik}")
                for ik in range(IK)
            ]

            # phase 1: hT = silu(w1.T @ xgT) * (w3.T @ xgT)
            # 2 * icpt * len(ch) == 8 PSUM accumulation groups per icg;
            # each weight tile is consumed within one hk iteration.
            for icg in range(n_icg):
                ps = {}
                for w in (0, 1):
                    for ic in range(icpt):
                        for ci, (off, sz) in enumerate(ch):
                            ps[w, ic, ci] = pspool.tile(
                                [128, sz], F32, tag="ps", name=f"ps{w}_{ic}_{ci}"
                            )
                if wt_w == 512:
                    # one weight tile per hk: 2KB/partition lines already
                    for hk in range(HK):
                        wt1 = wpool.tile(
                            [128, wt_w], F32R, tag="w1", name=f"w1t{hk}"
                        )
                        nc.sync.dma_start(wt1[:], w1[icg, hk // 2, :, hk % 2])
                        wt3 = wpool.tile(
                            [128, wt_w], F32R, tag="w3", name=f"w3t{hk}"
                        )
                        nc.sync.dma_start(wt3[:], w3[icg, hk // 2, :, hk % 2])
                        for w, wt in ((0, wt1), (1, wt3)):
                            for ic in range(icpt):
                                for ci, (off, sz) in enumerate(ch):
                                    nc.tensor.matmul(
                                        ps[w, ic, ci][:],
                                        wt[:, ic * 128 : (ic + 1) * 128],
                                        xg[hk][:, off : off + sz],
                                        start=(hk == 0),
                                        stop=(hk == HK - 1),
                                    )
                else:
                    # wt_w == 256: pair hk so the DMA block stays at
                    # 2KB/partition contiguous lines
                    for hk0 in range(0, HK, 2):
                        wt1 = wpool.tile(
                            [128, 2, wt_w], F32R, tag="w1", name=f"w1t{hk0}"
                        )
                        nc.sync.dma_start(wt1[:], w1[icg, hk0 // 2])
                        wt3 = wpool.tile(
                            [128, 2, wt_w], F32R, tag="w3", name=f"w3t{hk0}"
                        )
                        nc.sync.dma_start(wt3[:], w3[icg, hk0 // 2])
                        for hh in range(2):
                            hk = hk0 + hh
                            for w, wt in ((0, wt1), (1, wt3)):
                                for ic in range(icpt):
                                    for ci, (off, sz) in enumerate(ch):
                                        nc.tensor.matmul(
                                            ps[w, ic, ci][:],
                                            wt[:, hh, ic * 128 : (ic + 1) * 128],
                                            xg[hk][:, off : off + sz],
                                            start=(hk == 0),
                                            stop=(hk == HK - 1),
                                        )
                for ic in range(icpt):
                    ik = icg * icpt + ic
                    for ci, (off, sz) in enumerate(ch):
                        dst = hT[ik][:, off : off + sz]
                        nc.scalar.activation(dst, ps[0, ic, ci][:], SILU)
                        nc.vector.tensor_mul(dst, dst, ps[1, ic, ci][:])

            # phase 2: y = hT.T @ w2.  Process hg in pairs when PSUM
            # allows, so consecutive matmuls share the same stationary
            # operand.
            hg_grp = 2 if 2 * tt_n <= 8 else 1
            for hg0 in range(0, HG, hg_grp):
                hgs = list(range(hg0, hg0 + hg_grp))
                ps2 = {
                    (tt, hg): pspool.tile(
                        [128, 512], F32, tag="ps", name=f"ps2_{tt}_{hg}"
                    )
                    for tt in range(tt_n)
                    for hg in hgs
                }
                for ik in range(IK):
                    wts = {}
                    for hg in hgs:
                        wt = w2pool.tile(
                            [128, 512], F32R, tag=f"w2_{hg - hg0}",
                            name=f"w2t_{hg}_{ik}",
                        )
                        nc.sync.dma_start(wt[:], w2[hg, ik])
                        wts[hg] = wt
                    for tt in range(tt_n):
                        for hg in hgs:
                            nc.tensor.matmul(
                                ps2[tt, hg][:],
                                hT[ik][:, tt * 128 : (tt + 1) * 128],
                                wts[hg][:],
                                start=(ik == 0),
                                stop=(ik == IK - 1),
                            )
                for tt in range(tt_n):
                    for hg in hgs:
                        ot = opool.tile([128, 512], F32, tag="o", name=f"o{tt}_{hg}")
                        nc.vector.tensor_copy(ot[:], ps2[tt, hg][:])
                        nc.sync.dma_start(
                            y[tt * 128 : (tt + 1) * 128, hg * 512 : (hg + 1) * 512],
                            ot[:],
                        )
    nc.compile()
    return nc


def _get_prog(c):
    if c not in _prog_cache:
        _prog_cache[c] = _build(c)
    return _prog_cache[c]


def _retile_weights(w1, w2, w3, wt_w):
    """Host retiling so every device DMA is one contiguous block."""
    n_icg = I // wt_w
    # [E, n_icg, HK//2, 128, 2, wt_w]: (e, icg, hp, p, hh, i) =
    # w[e, (hp*2+hh)*128 + p, icg*wt_w + i]
    w1t = np.ascontiguousarray(
        w1.reshape(E, HK // 2, 2, 128, n_icg, wt_w).transpose(0, 4, 1, 3, 2, 5)
    )
    w3t = np.ascontiguousarray(
        w3.reshape(E, HK // 2, 2, 128, n_icg, wt_w).transpose(0, 4, 1, 3, 2, 5)
    )
    w2t = np.ascontiguousarray(
        w2.reshape(E, IK, 128, HG, 512).transpose(0, 3, 1, 2, 4)
    )
    return w1t, w3t, w2t


def kernel(x, expert_weights, w1, w2, w3, expert_indices):
    x = np.asarray(x, dtype=np.float32)
    expert_weights = np.asarray(expert_weights, dtype=np.float32)
    w1 = np.asarray(w1, dtype=np.float32)
    w2 = np.asarray(w2, dtype=np.float32)
    w3 = np.asarray(w3, dtype=np.float32)
    idx = np.asarray(expert_indices)
    T = x.shape[0]

    # Route: token lists per expert, merging duplicate top-k hits so each
    # token appears at most once per expert (scatter-add safe).
    same = idx[:, 0] == idx[:, 1]
    w_slot0 = np.where(same, expert_weights[:, 0] + expert_weights[:, 1],
                       expert_weights[:, 0])
    toks, wts = [], []
    for e in range(E):
        m0 = idx[:, 0] == e
        m1 = (idx[:, 1] == e) & ~same
        t0 = np.nonzero(m0)[0]
        t1 = np.nonzero(m1)[0]
        toks.append(np.concatenate([t0, t1]))
        wts.append(np.concatenate([w_slot0[m0], expert_weights[m1, 1]]))

    maxcount = max(len(t) for t in toks)
    maxcount = max(maxcount, 1)
    nrounds = -(-maxcount // C_CAP)
    c = -(-(-(-maxcount // nrounds)) // 128) * 128  # ceil to 128
    c = max(c, 128)

    w1t, w3t, w2t = _retile_weights(w1, w2, w3, _wtile(c))
    nc = _get_prog(c)

    out = np.zeros((T, H), dtype=np.float32)
    for r in range(nrounds):
        in_maps = []
        seg_toks = []
        seg_wts = []
        for e in range(E):
            seg = toks[e][r * c : (r + 1) * c]
            sw = wts[e][r * c : (r + 1) * c]
            seg_toks.append(seg)
            seg_wts.append(sw)
            xga = np.zeros((H, c), dtype=np.float32)
            if len(seg):
                xga[:, : len(seg)] = x[seg].T
            in_maps.append(
                {
                    "xgT": np.ascontiguousarray(xga.reshape(HK, 128, c)),
                    "w1": w1t[e],
                    "w3": w3t[e],
                    "w2": w2t[e],
                }
            )
        res = run_bass_kernel_spmd(nc, in_maps, core_ids=list(range(E)))
        for e in range(E):
            seg = seg_toks[e]
            if len(seg) == 0:
                continue
            ye = res.results[e]["y"][: len(seg)]
            out[seg] += ye * seg_wts[e][:, None]
    return out



# revision 2
# speedup vs baseline: 1.0031x; 1.0031x over previous
"""MoE SwiGLU experts (T=2048, H=2048, I=5632, E=8, top-2) on 8 trn2 cores.

Pair-parallel routed compute: experts are paired big-with-small (fold of
the sorted loads); the two NeuronCores of a pair each take HALF of the
intermediate dim I for BOTH experts of the pair, and the host sums the
two partial [c, H] outputs.  Every core runs identical-shape work of
(C1 + C2) tokens x I/2, so the matmul stream shrinks from 2 x max-load
to (biggest + 5th-biggest) expert loads — perfect balance inside a pair
regardless of routing.

f32r matmuls (HW: 227ns/MM at N=512 vs bf16 255 — f32r hides LDWEIGHTS
better).  xg ships bf16 and is cast to f32r by the idle Vector engine
(halves the startup HBM backlog); outputs return bf16.  Phase 1 uses 4
PSUM banks per 256-wide weight group so groups double-buffer across the
8 banks; phase 2 is output-transposed, one PSUM bank per 128-row H-tile.

Per core (pair (a,b), half m, segment capacities C1 >= C2):
  for seg in (a: C1, b: C2):
    phase 1: hT[i, t] = silu(w1h.T @ xgT) * (w3h.T @ xgT)   [I/2, c]
    phase 2: yT[h, t] = sum_ik w2h[ik, h].T @ hT[ik]        [H, c] partial
"""

import numpy as np
import ml_dtypes

import concourse.bacc as bacc
import concourse.mybir as mybir
import concourse.tile as tile
from concourse.bass_utils import run_bass_kernel_spmd

E = 8
H = 2048
I = 5632
IH = I // 2       # 2816: I-half per core
HK = H // 128     # 16 contraction chunks for phase 1
IKH = IH // 128   # 22 contraction chunks for phase 2 (per half)
HT = H // 128     # 16 output row tiles for phase 2
WT = 256          # phase-1 weight group width (I cols)
ICPT = WT // 128  # 2 ic-tiles per phase-1 weight group
NICG = IH // WT   # 11 phase-1 weight groups per half
HP = 4            # hk values packed per weight DMA tile

F32 = mybir.dt.float32
F32R = mybir.dt.float32r
BF16 = mybir.dt.bfloat16
NPBF = ml_dtypes.bfloat16
SILU = mybir.ActivationFunctionType.Silu

_prog_cache: dict[tuple, object] = {}


def _chunks(c):
    if c <= 512:
        return [(0, c)]
    c1 = -(-c // 16) * 8
    return [(0, c1), (c1, c - c1)]


def _build(C1, C2):
    nc = bacc.Bacc("TRN2", target_bir_lowering=False, debug=False, num_devices=E)
    caps = (C1, C2)
    xgT = [
        nc.dram_tensor(f"xg{s}", [HK, 128, caps[s]], BF16, kind="ExternalInput")
        for s in (0, 1)
    ]
    w1 = [
        nc.dram_tensor(f"w1{s}", [NICG, HP, 128, 4, WT], F32R, kind="ExternalInput")
        for s in (0, 1)
    ]
    w3 = [
        nc.dram_tensor(f"w3{s}", [NICG, HP, 128, 4, WT], F32R, kind="ExternalInput")
        for s in (0, 1)
    ]
    w2 = [
        nc.dram_tensor(f"w2{s}", [HT, 128, IKH, 128], F32R, kind="ExternalInput")
        for s in (0, 1)
    ]
    yT = [
        nc.dram_tensor(f"yT{s}", [HT, 128, caps[s]], BF16, kind="ExternalOutput")
        for s in (0, 1)
    ]
    scratch = nc.dram_tensor("scratch", [128, 256], F32R, kind="ExternalOutput")

    with tile.TileContext(nc) as tc:
        with (
            tc.tile_pool(name="xg", bufs=HK) as xpool,
            tc.tile_pool(name="stg", bufs=4) as stgpool,
            tc.tile_pool(name="h", bufs=IKH) as hpool,
            tc.tile_pool(name="w", bufs=6) as wpool,
            tc.tile_pool(name="w2p", bufs=3) as w2pool,
            tc.tile_pool(name="ps", bufs=8, space="PSUM") as pspool,
            tc.tile_pool(name="o", bufs=3) as opool,
            tc.tile_pool(name="wu", bufs=1) as wupool,
        ):
            # PE warmup on a zeroed f32 tile while the first DMAs land.
            wu0 = wupool.tile([128, 256], F32, tag="wu0", name="wu0")
            nc.vector.memset(wu0[:], 0.0)
            wups = pspool.tile([128, 256], F32, tag="ps", name="wups")
            for _ in range(5):
                nc.tensor.matmul(wups[:], wu0[:, :128], wu0[:], start=True, stop=True)
            wuo = wupool.tile([128, 256], F32R, tag="wuo", name="wuo")
            nc.vector.tensor_copy(wuo[:], wups[:])

            def emit_xg(s, hk):
                c = caps[s]
                st = stgpool.tile([128, c], BF16, tag="stg", name=f"stg{s}_{hk}")
                nc.scalar.dma_start(st[:], xgT[s][hk])
                t = xpool.tile([128, c], F32R, tag="xg", name=f"xg{s}_{hk}")
                nc.vector.tensor_copy(t[:], st[:])
                return t

            def emit_phase1(s, xg, hT, icg0_hook=None):
                c = caps[s]
                ch = _chunks(c)
                for icg in range(NICG):
                    ps = {}
                    for w in (0, 1):
                        for ic in range(ICPT):
                            for ci, (off, sz) in enumerate(ch):
                                ps[w, ic, ci] = pspool.tile(
                                    [128, sz], F32, tag="ps",
                                    name=f"ps{s}_{w}_{ic}_{ci}",
                                )
                    for hp in range(HP):
                        wt1 = wpool.tile(
                            [128, 4, WT], F32R, tag="w1", name=f"w1_{s}_{icg}_{hp}"
                        )
                        nc.sync.dma_start(wt1[:], w1[s][icg, hp])
                        wt3 = wpool.tile(
                            [128, 4, WT], F32R, tag="w3", name=f"w3_{s}_{icg}_{hp}"
                        )
                        nc.sync.dma_start(wt3[:], w3[s][icg, hp])
                        if icg == 0 and icg0_hook:
                            icg0_hook(hp)
                        for hh in range(4):
                            hk = hp * 4 + hh
                            for w, wt in ((0, wt1), (1, wt3)):
                                for ic in range(ICPT):
                                    for ci, (off, sz) in enumerate(ch):
                                        nc.tensor.matmul(
                                            ps[w, ic, ci][:],
                                            wt[:, hh, ic * 128 : (ic + 1) * 128],
                                            xg[hk][:, off : off + sz],
                                            start=(hk == 0),
                                            stop=(hk == HK - 1),
                                        )
                    for ic in range(ICPT):
                        ik = icg * ICPT + ic
                        for ci, (off, sz) in enumerate(ch):
                            dst = hT[ik][:, off : off + sz]
                            nc.scalar.activation(dst, ps[0, ic, ci][:], SILU)
                            nc.vector.tensor_mul(dst, dst, ps[1, ic, ci][:])

            def emit_phase2(s, hT, prefetch):
                c = caps[s]
                ch = _chunks(c)
                for h in range(HT):
                    w2t = w2pool.tile(
                        [128, IKH, 128], F32R, tag="w2", name=f"w2_{s}_{h}"
                    )
                    nc.scalar.dma_start(w2t[:], w2[s][h])
                    for ci, (off, sz) in enumerate(ch):
                        ps2 = pspool.tile(
                            [128, sz], F32, tag="ps", name=f"ps2_{s}_{h}_{ci}"
                        )
                        for ik in range(IKH):
                            nc.tensor.matmul(
                                ps2[:],
                                w2t[:, ik, :],
                                hT[ik][:, off : off + sz],
                                start=(ik == 0),
                                stop=(ik == IKH - 1),
                            )
                        ot = opool.tile([128, sz], BF16, tag="o", name=f"o{s}_{h}_{ci}")
                        nc.vector.tensor_copy(ot[:], ps2[:])
                        nc.scalar.dma_start(yT[s][h, :, off : off + sz], ot[:])
                    if prefetch and h < 2 * HK and h % 2 == 0:
                        # stage segment 1's tokens during segment 0's phase 2
                        prefetch(h // 2)

            # Stage the first half of segment 0's tokens up front; the
            # rest interleaves into icg 0's weight stream so the xg traffic
            # does not crowd the first weight tiles off the HBM bus.
            xg0 = [None] * HK
            for hk in range(8):
                xg0[hk] = emit_xg(0, hk)

            def stage_seg0(hp):
                for hk in (8 + 2 * hp, 9 + 2 * hp):
                    xg0[hk] = emit_xg(0, hk)

            hT0 = [
                hpool.tile([128, C1], F32R, tag="h", name=f"h0_{ik}")
                for ik in range(IKH)
            ]
            emit_phase1(0, xg0, hT0, icg0_hook=stage_seg0)

            xg1 = [None] * HK

            def stage_seg1(hk):
                xg1[hk] = emit_xg(1, hk)

            emit_phase2(0, hT0, stage_seg1)
            for hk in range(HK):
                if xg1[hk] is None:
                    xg1[hk] = emit_xg(1, hk)

            hT1 = [
                hpool.tile([128, C2], F32R, tag="h", name=f"h1_{ik}")
                for ik in range(IKH)
            ]
            emit_phase1(1, xg1, hT1)
            emit_phase2(1, hT1, None)

            nc.scalar.dma_start(scratch[:], wuo[:])
    nc.compile()
    return nc


def _get_prog(C1, C2):
    key = (C1, C2)
    if key not in _prog_cache:
        _prog_cache[key] = _build(C1, C2)
    return _prog_cache[key]


def _retile_weights(w1, w2, w3):
    """Retile into per-(expert, I-half) contiguous blocks."""
    w1 = np.asarray(w1, np.float32)
    w3 = np.asarray(w3, np.float32)
    w2 = np.asarray(w2, np.float32)
    # [E, 2, NICG, HP, 128, 4, WT]:
    # (e,half,icg,hp,p,hh,i) = w[e, (hp*4+hh)*128+p, half*IH + icg*WT + i]
    w1t = np.ascontiguousarray(
        w1.reshape(E, HP, 4, 128, 2, NICG, WT).transpose(0, 4, 5, 1, 3, 2, 6)
    )
    w3t = np.ascontiguousarray(
        w3.reshape(E, HP, 4, 128, 2, NICG, WT).transpose(0, 4, 5, 1, 3, 2, 6)
    )
    # [E, 2, HT, 128, IKH, 128]:
    # (e,half,h,p,ik,j) = w2[e, half*IH + ik*128 + p, h*128 + j]
    w2t = np.ascontiguousarray(
        w2.reshape(E, 2, IKH, 128, HT, 128).transpose(0, 1, 4, 3, 2, 5)
    )
    return w1t, w3t, w2t


def _gather_bf16(x, seg, c):
    xga = np.zeros((H, c), dtype=np.float32)
    if len(seg):
        xga[:, : len(seg)] = x[seg].T
    return np.ascontiguousarray(xga.astype(NPBF).reshape(HK, 128, c))


def kernel(x, expert_weights, w1, w2, w3, expert_indices):
    x = np.asarray(x, dtype=np.float32)
    expert_weights = np.asarray(expert_weights, dtype=np.float32)
    idx = np.asarray(expert_indices)
    T = x.shape[0]

    # Route: token lists per expert, merging duplicate top-k hits.
    same = idx[:, 0] == idx[:, 1]
    w_slot0 = np.where(same, expert_weights[:, 0] + expert_weights[:, 1],
                       expert_weights[:, 0])
    toks, wts = [], []
    for e in range(E):
        m0 = idx[:, 0] == e
        m1 = (idx[:, 1] == e) & ~same
        t0 = np.nonzero(m0)[0]
        t1 = np.nonzero(m1)[0]
        toks.append(np.concatenate([t0, t1]))
        wts.append(np.concatenate([w_slot0[m0], expert_weights[m1, 1]]))

    counts = np.array([len(t) for t in toks])
    order = np.argsort(-counts)          # experts by load, descending
    bigs = [int(order[k]) for k in range(4)]
    smalls = [int(order[7 - k]) for k in range(4)]

    nrounds = max(1, -(-int(counts.max()) // 1024))
    C1 = max(128, -(-(-(-max(counts[b] for b in bigs) // nrounds)) // 8) * 8)
    C2 = max(128, -(-(-(-max(counts[s] for s in smalls) // nrounds)) // 8) * 8)

    w1t, w3t, w2t = _retile_weights(w1, w2, w3)
    nc = _get_prog(C1, C2)

    out = np.zeros((T, H), dtype=np.float32)
    for r in range(nrounds):
        in_maps = [None] * E
        seg_of_pair = []
        for k in range(4):
            b, s = bigs[k], smalls[k]
            segb = toks[b][r * C1 : (r + 1) * C1]
            segs = toks[s][r * C2 : (r + 1) * C2]
            wb = wts[b][r * C1 : (r + 1) * C1]
            ws = wts[s][r * C2 : (r + 1) * C2]
            seg_of_pair.append((segb, wb, segs, ws))
            xga = _gather_bf16(x, segb, C1)
            xgb = _gather_bf16(x, segs, C2)
            for half in (0, 1):
                in_maps[2 * k + half] = {
                    "xg0": xga,
                    "xg1": xgb,
                    "w10": w1t[b, half],
                    "w30": w3t[b, half],
                    "w20": w2t[b, half],
                    "w11": w1t[s, half],
                    "w31": w3t[s, half],
                    "w21": w2t[s, half],
                }
        res = run_bass_kernel_spmd(nc, in_maps, core_ids=list(range(E)))
        for k in range(4):
            segb, wb, segs, ws = seg_of_pair[k]
            r0 = res.results[2 * k]
            r1 = res.results[2 * k + 1]
            if len(segb):
                ya = (r0["yT0"].astype(np.float32) + r1["yT0"].astype(np.float32))
                ya = ya.reshape(H, C1).T[: len(segb)]
                out[segb] += ya * wb[:, None]
            if len(segs):
                yb = (r0["yT1"].astype(np.float32) + r1["yT1"].astype(np.float32))
                yb = yb.reshape(H, C2).T[: len(segs)]
                out[segs] += yb * ws[:, None]
    return out
